# revision 7
# baseline (speedup 1.0000x reference)
"""Boundaries-loss kernel v3: exact IVF-style pruning + v2 drain.

Per batch element (one NeuronCore):
  - Host kd-sorts the masked boundary samples into spatially-local 128-sample
    tiles and the verts into 80 kd-leaves of 128.
  - For each sample tile, host keeps only vert leaves whose bounding-box
    lower-bound distance can beat a per-sample upper bound (nearest of 512
    sampled verts).  Exact: every sample's true nearest vert is in its
    tile's kept set.
  - Kept leaves are packed into per-tile slot blocks of 16 leaves
    (4 strips x 512 cols) in DRAM; the device streams them per group.
    The group counts per tile are shared across cores (max), keeping the
    program SPMD; slot contents are per-core.
  - Device: 4x row-tiled matmuls (K=24 in 32-row strips) produce a 4-bank
    group; group 0 drains via direct fp32 tensor_reduce from PSUM, other
    groups via ACT fp16 cast + DVE 2x min tree (v2 drain).
"""

import os
import sys
from contextlib import ExitStack

import numpy as np

for _p in ("/opt/trn_rl_repo", "/root/.axon_site/_ro/trn_rl_repo"):
    if os.path.isdir(_p) and _p not in sys.path:
        sys.path.append(_p)

import ml_dtypes

BT, NV, NB, NS = 8, 10000, 16384, 4096
VT = 512
NSTRIP = 4
LEAF = 64
NVP = 10240
NL = NVP // LEAF          # kd leaves
LPS = VT // LEAF          # leaves per strip-bank
LPG = NSTRIP * LPS        # leaves per group
K = 24
NREP = 512

_PAIRS = [(0, 0), (0, 1), (1, 0), (0, 2), (2, 0), (1, 1)]
_BF16 = ml_dtypes.bfloat16

_COMPILED = {}
_LAST_EXEC_NS = None


def _bf16_split3(x):
    p0 = x.astype(_BF16)
    r = x - p0.astype(np.float32)
    p1 = r.astype(_BF16)
    r = r - p1.astype(np.float32)
    p2 = r.astype(_BF16)
    return p0, p1, p2


def _kd_split(o, pts, sizes):
    if len(sizes) == 1:
        return [o]
    h = len(sizes) // 2
    n_left = sum(sizes[:h])
    ext = pts[o].max(0) - pts[o].min(0)
    d = int(np.argmax(ext))
    oo = o[np.argsort(pts[o, d], kind="stable")]
    return _kd_split(oo[:n_left], pts, sizes[:h]) + _kd_split(oo[n_left:], pts, sizes[h:])


def _build_program(S, ngroups):
    """ngroups: tuple of per-tile group counts (shared across cores)."""
    import concourse.bass as bass  # noqa: F401
    import concourse.tile as tile
    from concourse import bacc, mybir

    T = S // 128
    assert len(ngroups) == T
    NSLOT = sum(ngroups)
    dt = mybir.dt
    mn = mybir.AluOpType.min
    nc = bacc.Bacc(
        "TRN2",
        target_bir_lowering=False,
        debug=False,
        enable_asserts=False,
        num_devices=BT,
    )

    lhsT_p = nc.dram_tensor("lhsT_p", [K, S], dt.bfloat16, kind="ExternalInput").ap()
    rhs_s = nc.dram_tensor(
        "rhs_s", [NSLOT * 128, VT], dt.bfloat16, kind="ExternalInput"
    ).ap()
    msk = nc.dram_tensor("msk", [128, T], dt.float32, kind="ExternalInput").ap()
    out = nc.dram_tensor("out", [128, 1], dt.float32, kind="ExternalOutput").ap()

    with tile.TileContext(nc) as tc, ExitStack() as ctx:
        const = ctx.enter_context(tc.tile_pool(name="const", bufs=1))
        psum = ctx.enter_context(tc.tile_pool(name="psum", bufs=2, space="PSUM"))
        trash_pool = ctx.enter_context(tc.tile_pool(name="trash", bufs=2))
        cast_pool = ctx.enter_context(tc.tile_pool(name="cast", bufs=4))
        rhs_pool = ctx.enter_context(tc.tile_pool(name="rhs", bufs=8))

        lhsT_sb = const.tile([128, S], dt.bfloat16)
        # head chunk first so tile 0's matmuls start early; the tail streams
        # in behind the first slot DMAs.
        nc.scalar.dma_start(out=lhsT_sb[0:K, 0:128], in_=lhsT_p[:, 0:128])
        for i in range(1, NSTRIP):
            nc.scalar.dma_start(
                out=lhsT_sb[32 * i : 32 * i + K, 0:128], in_=lhsT_sb[0:K, 0:128]
            )
        msk_sb = const.tile([128, T], dt.float32)
        nc.scalar.dma_start(out=msk_sb[:], in_=msk)

        mins2 = const.tile([128, 2 * T], dt.float32)
        nc.vector.memset(mins2[:], 3.0e38)

        slot = 0
        for t in range(T):
            n_t = ngroups[t]
            # the direct PSUM reduce costs the DVE ~2.3us/tile; with pruned
            # group counts the ACT has slack, so most tiles go all-cast and
            # only every third tile keeps the direct path (ACT/DVE balance).
            use_direct = t % 3 == 0
            c16s = []
            for g in range(n_t):
                rg = rhs_pool.tile([128, VT], dt.bfloat16, tag="rg")
                eng = nc.sync if slot % 2 == 0 else nc.scalar
                eng.dma_start(
                    out=rg[:], in_=rhs_s[slot * 128 : (slot + 1) * 128, :]
                )
                if slot == 0 and S > 128:
                    nc.sync.dma_start(
                        out=lhsT_sb[0:K, 128:S], in_=lhsT_p[:, 128:S]
                    )
                    for i in range(1, NSTRIP):
                        eng2 = nc.scalar if i % 2 else nc.sync
                        eng2.dma_start(
                            out=lhsT_sb[32 * i : 32 * i + K, 128:S],
                            in_=lhsT_sb[0:K, 128:S],
                        )
                pq = psum.tile([128, NSTRIP * VT], dt.float32, tag="quad")
                for i in range(NSTRIP):
                    nc.tensor.matmul(
                        pq[:, i * VT : (i + 1) * VT],
                        lhsT_sb[32 * i : 32 * i + K, t * 128 : (t + 1) * 128],
                        rg[32 * i : 32 * i + K, :],
                        tile_position=(32 * i, 0),
                    )
                slot += 1
                if g > 0 or not use_direct:
                    c16 = cast_pool.tile([128, 4 * VT], dt.float16, tag=f"c{g}")
                    nc.scalar.copy(c16[:], pq[:])
                    c16s.append(c16)
                else:
                    with tc.high_priority(offset=45):
                        nc.vector.tensor_reduce(
                            mins2[:, 2 * t + 1 : 2 * t + 2],
                            pq[:],
                            axis=mybir.AxisListType.X,
                            op=mn,
                        )
            # fp16 2x min tree over the casts (0..n_t-1 of them)
            if c16s:
                m = c16s[0]
                for j in range(1, len(c16s)):
                    nx = trash_pool.tile([128, 4 * VT], dt.float16, tag=f"m{j % 2}")
                    nc.vector.tensor_tensor(out=nx[:], in0=m[:], in1=c16s[j][:], op=mn)
                    m = nx
                f1 = trash_pool.tile([128, 2 * VT], dt.float16, tag="f1")
                nc.vector.tensor_tensor(
                    out=f1[:], in0=m[:, 0 : 2 * VT], in1=m[:, 2 * VT : 4 * VT], op=mn
                )
                f2 = trash_pool.tile([128, VT], dt.float16, tag="f2")
                nc.vector.tensor_tensor(
                    out=f2[:], in0=f1[:, 0:VT], in1=f1[:, VT : 2 * VT], op=mn
                )
                nc.vector.tensor_reduce(
                    mins2[:, 2 * t : 2 * t + 1],
                    f2[:],
                    axis=mybir.AxisListType.X,
                    op=mn,
                )

        mins = const.tile([128, T], dt.float32)
        nc.vector.tensor_reduce(
            mins[:],
            mins2[:].rearrange("p (t g) -> p t g", g=2),
            axis=mybir.AxisListType.X,
            op=mn,
        )
        masked = const.tile([128, T], dt.float32)
        nc.vector.tensor_mul(masked[:], mins[:], msk_sb[:])
        col = const.tile([128, 1], dt.float32)
        nc.vector.tensor_reduce(
            col[:], masked[:], axis=mybir.AxisListType.X, op=mybir.AluOpType.add
        )
        nc.sync.dma_start(out=out, in_=col[:])

    nc.compile()
    return nc


def _features(coords_pad, verts_leafordered):
    """lhsT [K, S] and rhs [K, NVP] in leaf order."""
    S = coords_pad.shape[0]
    sqb = np.sum(coords_pad * coords_pad, axis=-1, dtype=np.float32)
    b_parts = _bf16_split3(coords_pad)
    w = (-2.0 * verts_leafordered).astype(np.float32)
    sqv = np.sum(verts_leafordered * verts_leafordered, axis=-1, dtype=np.float32)
    w_parts = _bf16_split3(w)
    s_parts = _bf16_split3(sqv)
    lhsT = np.empty((K, S), dtype=_BF16)
    rhs = np.empty((K, NVP), dtype=_BF16)
    for d in range(3):
        for r, (i, j) in enumerate(_PAIRS):
            lhsT[6 * d + r] = b_parts[i][:, d]
            rhs[6 * d + r] = w_parts[j][:, d]
    for j in range(3):
        lhsT[18 + j] = np.ones((S,), dtype=_BF16)
        rhs[18 + j] = s_parts[j]
    sqb_parts = _bf16_split3(sqb)
    for j in range(3):
        lhsT[21 + j] = sqb_parts[j]
        rhs[21 + j] = np.ones((NVP,), dtype=_BF16)
    return lhsT, rhs


def _prepare_all(verts, bds, indices):
    verts = np.asarray(verts, dtype=np.float32)
    bds = np.asarray(bds, dtype=np.float32)
    idx = np.asarray(indices).astype(np.int64)
    rng = np.random.default_rng(12345)

    bsel = bds[:, idx, :]
    coords_all = bsel[..., :3]
    m_all = bsel[..., 3]

    per_core = []
    max_T = 0
    for b in range(BT):
        act = np.nonzero(m_all[b] != 0.0)[0]
        na = len(act)
        if na == 0:
            per_core.append(None)
            continue
        coords = coords_all[b][act].astype(np.float32)
        mvals = m_all[b][act].astype(np.float32)
        T = (na + 127) // 128
        max_T = max(max_T, T)
        base, rem = divmod(na, T)
        sizes = [base + (1 if i < rem else 0) for i in range(T)]
        sleaves = _kd_split(np.arange(na), coords, sizes)
        per_core.append((coords, mvals, T, sleaves))
    if max_T == 0:
        return None, None, None
    S = max_T * 128
    T = max_T

    # per-core kd leaves over verts + kept sets
    kept_all = []
    packs = []
    for b in range(BT):
        if per_core[b] is None:
            packs.append(None)
            kept_all.append([[0]] * T)
            continue
        coords, mvals, T_b, sleaves = per_core[b]
        pad = np.zeros((S, 3), np.float32)
        mpad = np.zeros((S,), np.float32)
        pos = 0
        for t, L in enumerate(sleaves):
            pad[t * 128 : t * 128 + len(L)] = coords[L]
            mpad[t * 128 : t * 128 + len(L)] = mvals[L]
        vp = np.zeros((NVP, 3), np.float32)
        vp[:NV] = verts[b]
        vp[NV:] = 1.0e6
        vleaves = _kd_split(np.arange(NVP), vp, [LEAF] * NL)
        vorder = np.concatenate(vleaves)
        vlo = np.stack([vp[L].min(0) for L in vleaves])
        vhi = np.stack([vp[L].max(0) for L in vleaves])
        vleaf_pts = vp[np.stack(vleaves)]  # [NL, LEAF, 3]
        kept_b = []
        for t in range(T):
            ss = pad[t * 128 : (t + 1) * 128]
            live = np.zeros(128, bool)
            if t < T_b:
                live[: len(sleaves[t])] = True
            if not live.any():
                kept_b.append([0])
                continue
            sl = ss[live]
            dlo = np.maximum(vlo[None] - sl[:, None], 0)
            dhi = np.maximum(sl[:, None] - vhi[None], 0)
            lb2 = ((dlo + dhi) ** 2).sum(-1)
            # tight per-sample upper bound: exact min over the 3 leaves with
            # the smallest box lower bound (an upper bound of the true min)
            near = np.argpartition(lb2, 3, axis=1)[:, :3]
            cand = vleaf_pts[near]  # [ns, 3, LEAF, 3]
            ub = ((cand - sl[:, None, None, :]) ** 2).sum(-1).min((1, 2))
            keep = (lb2 <= ub[:, None] * (1 + 1e-5) + 1e-20).any(0)
            kl = np.nonzero(keep)[0].tolist()
            if not kl:
                kl = [0]
            kept_b.append(kl)
        order = sorted(range(T), key=lambda t: -len(kept_b[t]))
        kept_b = [kept_b[t] for t in order]
        pad = np.concatenate([pad[t * 128 : (t + 1) * 128] for t in order])
        mpad = np.concatenate([mpad[t * 128 : (t + 1) * 128] for t in order])
        kept_all.append(kept_b)
        packs.append((pad, mpad, vp[vorder]))

    ngroups = tuple(
        max(
            (len(kept_all[b][t]) + LPG - 1) // LPG if packs[b] is not None else 1
            for b in range(BT)
        )
        for t in range(T)
    )
    NSLOT = sum(ngroups)

    in_maps = []
    for b in range(BT):
        if packs[b] is None:
            in_maps.append(
                {
                    "lhsT_p": np.zeros((K, S), dtype=_BF16),
                    "rhs_s": np.zeros((K, NSLOT * NSTRIP * VT), dtype=_BF16),
                    "msk": np.zeros((128, T), dtype=np.float32),
                }
            )
            continue
        pad, mpad, vlead = packs[b]
        lhsT, rhs = _features(pad, vlead)
        # [slot, strip(32-row padded), VT] blocks: one plain [128, VT] DMA
        # per group, rows 32s..32s+K carry strip s's four leaves.
        rhs_s = np.zeros((NSLOT * 128, VT), dtype=_BF16)
        slot = 0
        for t in range(T):
            kl = kept_all[b][t]
            n_t = ngroups[t]
            need = n_t * LPG
            kl_pad = (kl * ((need // len(kl)) + 1))[:need]
            for g in range(n_t):
                for s in range(NSTRIP):
                    row = (slot + g) * 128 + 32 * s
                    blk = np.concatenate(
                        [
                            rhs[:, leaf * LEAF : (leaf + 1) * LEAF]
                            for leaf in kl_pad[
                                g * LPG + LPS * s : g * LPG + LPS * (s + 1)
                            ]
                        ],
                        axis=1,
                    )
                    rhs_s[row : row + K, :] = blk
            slot += n_t
        in_maps.append(
            {
                "lhsT_p": np.ascontiguousarray(lhsT),
                "rhs_s": np.ascontiguousarray(rhs_s),
                "msk": np.ascontiguousarray(mpad.reshape(T, 128).T),
            }
        )
    return S, ngroups, in_maps


def _ensure_ntff_hook():
    import types

    try:
        from antenv.axon_hooks import get_axon_ntff_profile_hook  # noqa: F401

        return True
    except ImportError:
        pass
    try:
        import antenv
        from trn_agent_boot.trn_boot import _ntff_profile_via_ctypes

        hook = _ntff_profile_via_ctypes("/opt/axon/libaxon_pjrt.so")
        if hook is None:
            return False
        mod = types.ModuleType("antenv.axon_hooks")
        mod.get_axon_ntff_profile_hook = lambda: hook
        mod.set_axon_ntff_profile_hook = lambda h: None
        sys.modules["antenv.axon_hooks"] = mod
        antenv.axon_hooks = mod
        return True
    except Exception:
        return False


def kernel(verts, bds, pix_to_face, indices):
    global _LAST_EXEC_NS
    S, ngroups, in_maps = _prepare_all(verts, bds, indices)
    if S is None:
        return np.float32(0.0)

    key = (S, ngroups)
    if key not in _COMPILED:
        _COMPILED[key] = _build_program(S, ngroups)
    nc = _COMPILED[key]

    from concourse import bass_utils

    trace = os.environ.get("BOUNDARIES_TRACE", "0") == "1" and _ensure_ntff_hook()
    if trace:
        bass_utils.upload_artifacts = lambda tmpdir: "local://unused"

    try:
        res = bass_utils.run_bass_kernel_spmd(
            nc, in_maps, core_ids=list(range(BT)), trace=trace
        )
    except Exception:
        if not trace:
            raise
        res = bass_utils.run_bass_kernel_spmd(
            nc, in_maps, core_ids=list(range(BT)), trace=False
        )
    _LAST_EXEC_NS = res.exec_time_ns

    total = sum(
        float(np.sum(res.results[b]["out"].astype(np.float64))) for b in range(BT)
    )
    return np.float32(total / (NS * BT))


if __name__ == "__main__":
    rng = np.random.default_rng(0)
    verts = rng.standard_normal((BT, NV, 3), dtype=np.float32)
    bds = rng.standard_normal((BT, NB, 4), dtype=np.float32)
    bds[..., 3] = (rng.random((BT, NB)) > 0.5).astype(np.float32)
    pix = np.zeros((BT, 256, 256, 1), dtype=np.int32)
    idx = rng.permutation(NB)[:NS].astype(np.int64)

    bv = bds[:, idx, :3]
    bm = bds[:, idx, 3]
    d = (
        np.sum(bv * bv, -1)[:, :, None]
        + np.sum(verts * verts, -1)[:, None, :]
        - 2.0 * np.einsum("bsd,bvd->bsv", bv, verts)
    )
    expected = np.mean(np.min(d, -1) * bm)

    actual = kernel(verts, bds, pix, idx)
    rel = abs(actual - expected) / max(abs(expected), 1e-12)
    print(f"expected={expected:.8f} actual={actual:.8f} rel={rel:.3e}")
